# revision 11
# baseline (speedup 1.0000x reference)
"""Distributed Trainium2 kernel for 16-head causal attention with RoPE.

B=2, S=2048, D=2048, H=16, HD=128. Tensor-parallel over heads: core c owns
heads {2c, 2c+1}. Each core computes q/k/v projections for its heads,
RoPE, causal attention, and a partial output projection (wo row-shard);
the host sums the 8 partials (the unshard step for a row-sharded wo).

Device-side layout choices (all transposes are done on the host):
  - x is fed pre-transposed as xt[d, tok] so every matmul contracts over
    the partition axis with no on-device transposes.
  - q/k are produced head-dim-major (qT[hd, tok]); the RoPE even/odd pair
    permutation is folded into the wq/wk columns on the host, so RoPE is
    six plain elementwise ops on [64, tok] slices.
  - scores are computed transposed (scoresT[k, q]); softmax sums over k
    (the partition axis) come from an all-ones [128,128] matmul that
    also broadcasts the sum to all partitions; 1/sum = exp(-ln(sum)).
  - attention output oT[hd, q] is exactly the lhsT the output projection
    needs, so the whole pipeline has zero on-device transposes.
"""

import numpy as np
from contextlib import ExitStack

B, S, D = 2, 2048, 2048
H, HD, HALF = 16, 128, 64
BS = B * S
NCORES = 8
HPC = H // NCORES          # heads per core
TT = 512                   # token tile for projections
QT = 512                   # q tile in attention
KC = 128                   # k chunk in attention
NKT = D // 128             # 16 contraction chunks of the model dim
NTT = BS // TT             # 8 token tiles
ISQRT = 1.0 / float(np.sqrt(HD))


def _legalize_waits(bir: bytes) -> bytes:
    """Split multi-wait sync_info into standalone EventSemaphore instructions.

    The neuronxcc walrus codegen only encodes ONE sync wait slot on compute
    instructions (Matmult/TensorTensor/...); Tile's sem-assignment freely
    emits several. Hoisting the extras into same-engine EventSemaphore
    instructions placed immediately before the consumer is semantically
    identical (the sequencer blocks on them in program order).
    """
    import json

    d = json.loads(bir)
    wid = 0
    for fn in d["functions"]:
        for blk in fn["blocks"]:
            out = []
            for inst in blk["instructions"]:
                si = inst.get("sync_info")
                if si:
                    waits = si.get("on_wait") or []
                    if len(waits) > 1 and inst.get("engine") not in (None, "Unassigned"):
                        for w in waits[:-1]:
                            wid += 1
                            out.append(
                                {
                                    "debug": inst.get("debug", 0),
                                    "engine": inst["engine"],
                                    "ins": [],
                                    "name": f"hoisted-wait-{wid}",
                                    "opcode": "EventSemaphore",
                                    "outs": [],
                                    "sync_info": {"on_update": [], "on_wait": [w]},
                                }
                            )
                        si["on_wait"] = [waits[-1]]
                out.append(inst)
            blk["instructions"] = out
    return json.dumps(d).encode()


def _patch_serialization(nc):
    import types

    orig = nc.to_json_bytes

    def patched(self):
        return _legalize_waits(orig())

    nc.to_json_bytes = types.MethodType(patched, nc)
    return nc


def _build_nc():
    import concourse.bass as bass
    import concourse.tile as tile
    from concourse import mybir

    f32 = mybir.dt.float32
    bf16 = mybir.dt.bfloat16
    Exp = mybir.ActivationFunctionType.Exp
    Ln = mybir.ActivationFunctionType.Ln
    mult = mybir.AluOpType.mult
    sub = mybir.AluOpType.subtract
    add = mybir.AluOpType.add

    nc = bass.Bass()

    xt_h = nc.declare_dram_parameter("xt", [128, NKT, BS], bf16, isOutput=False)
    wq_h = nc.declare_dram_parameter("wq", [128, NKT, 2 * HD], bf16, isOutput=False)
    wk_h = nc.declare_dram_parameter("wk", [128, NKT, 2 * HD], bf16, isOutput=False)
    wv_h = nc.declare_dram_parameter("wv", [128, NKT, 2 * HD], bf16, isOutput=False)
    wo_h = nc.declare_dram_parameter("wo", [128, 2, D], bf16, isOutput=False)
    cs_h = nc.declare_dram_parameter("cs", [128, 2 * BS], bf16, isOutput=False)
    m4_h = nc.declare_dram_parameter("m4", [128, 4 * QT], bf16, isOutput=False)
    out_h = nc.declare_dram_parameter("out", [BS, D], f32, isOutput=True)

    with ExitStack() as ctx:
        tc = ctx.enter_context(tile.TileContext(nc))
        const = ctx.enter_context(tc.tile_pool(name="const", bufs=1))
        persist = ctx.enter_context(tc.tile_pool(name="persist", bufs=1))
        xtp = ctx.enter_context(tc.tile_pool(name="xtp", bufs=2))
        expp = ctx.enter_context(tc.tile_pool(name="expp", bufs=4))
        esp = ctx.enter_context(tc.tile_pool(name="esp", bufs=4))
        ropet = ctx.enter_context(tc.tile_pool(name="ropet", bufs=8))
        fpool = ctx.enter_context(tc.tile_pool(name="fpool", bufs=3))
        outp = ctx.enter_context(tc.tile_pool(name="outp", bufs=4))
        psA = ctx.enter_context(tc.tile_pool(name="psA", bufs=2, space="PSUM"))
        psS = ctx.enter_context(tc.tile_pool(name="psS", bufs=2, space="PSUM"))
        psO = ctx.enter_context(tc.tile_pool(name="psO", bufs=1, space="PSUM"))
        psN = ctx.enter_context(tc.tile_pool(name="psN", bufs=1, space="PSUM"))

        # ---- constants into SBUF (fine-grained DMAs so the first
        # projection matmuls start as soon as their slices land) ----
        wq_sb = const.tile([128, NKT, 2 * HD], bf16, tag="wq")
        wk_sb = const.tile([128, NKT, 2 * HD], bf16, tag="wk")
        wv_sb = const.tile([128, NKT, 2 * HD], bf16, tag="wv")
        wo_sb = const.tile([128, 2, D], bf16, tag="wo")
        cs_sb = const.tile([128, 2 * BS], bf16, tag="cs")
        m4_sb = const.tile([128, 4 * QT], bf16, tag="m4")
        ones_sb = const.tile([128, 128], bf16, tag="ones")
        for c in range(NKT):
            nc.sync.dma_start(wq_sb[:, c, :], wq_h[:, c, :])
        for c in range(NKT):
            nc.sync.dma_start(wk_sb[:, c, :], wk_h[:, c, :])
        for c in range(NKT):
            nc.sync.dma_start(wv_sb[:, c, :], wv_h[:, c, :])
        nc.sync.dma_start(cs_sb[:], cs_h[:])
        nc.sync.dma_start(m4_sb[:], m4_h[:])
        nc.sync.dma_start(wo_sb[:], wo_h[:])
        nc.vector.memset(ones_sb[:], 1.0)

        # DVE pre-touch of DMA-written constants: TensorTensor instructions
        # encode only one sync-wait slot, so the DVE vector clock must have
        # observed these DMAs before any TT reads them (else walrus dies with
        # "Too many sync wait commands").
        scratch = const.tile([1, 8], bf16, tag="scratch")
        nc.vector.tensor_copy(scratch[0:1, 0:2], cs_sb[0:1, 0:2])
        nc.vector.tensor_copy(scratch[0:1, 2:4], m4_sb[0:1, 0:2])

        # persistent activations
        qr = persist.tile([128, HPC, BS], bf16, tag="qr")   # rotated qT per head
        kr = persist.tile([128, HPC, BS], bf16, tag="kr")   # rotated kT per head
        v_sb = persist.tile([128, BS // 128, 2 * HD], bf16, tag="v")  # tok-major v
        on_sb = persist.tile([128, HPC, B, S], bf16, tag="on")  # normalized oT

        # ---- phase 1: projections + RoPE ----
        for t in range(NTT):
            t0 = t * TT
            xt_t = xtp.tile([128, NKT, TT], bf16, tag="xt")
            for c in range(NKT):
                nc.sync.dma_start(xt_t[:, c, :], xt_h[:, c, t0 : t0 + TT])

            for h in range(HPC):
                for w_sb, dstT in ((wq_sb, qr), (wk_sb, kr)):
                    pq = psA.tile([128, TT], mybir.dt.float32, tag="proj")
                    for c in range(NKT):
                        nc.tensor.matmul(
                            pq[:],
                            w_sb[:, c, h * HD : (h + 1) * HD],
                            xt_t[:, c, :],
                            start=(c == 0),
                            stop=(c == NKT - 1),
                        )
                    co = cs_sb[0:HALF, h * BS + t0 : h * BS + t0 + TT]
                    si = cs_sb[HALF:128, h * BS + t0 : h * BS + t0 + TT]
                    t1 = ropet.tile([HALF, TT], bf16, tag="rt")
                    t2 = ropet.tile([HALF, TT], bf16, tag="rt")
                    t3 = ropet.tile([HALF, TT], bf16, tag="rt")
                    t4 = ropet.tile([HALF, TT], bf16, tag="rt")
                    nc.vector.tensor_tensor(t1[:], pq[0:HALF, :], co, mult)
                    nc.vector.tensor_tensor(t2[:], pq[HALF:128, :], si, mult)
                    nc.vector.tensor_tensor(
                        dstT[0:HALF, h, t0 : t0 + TT], t1[:], t2[:], sub
                    )
                    nc.vector.tensor_tensor(t3[:], pq[0:HALF, :], si, mult)
                    nc.vector.tensor_tensor(t4[:], pq[HALF:128, :], co, mult)
                    nc.vector.tensor_tensor(
                        dstT[HALF:128, h, t0 : t0 + TT], t3[:], t4[:], add
                    )

            # v projection, token-major [tok, 2*HD]
            for m in range(TT // 128):
                pv = psA.tile([128, 2 * HD], mybir.dt.float32, tag="proj")
                for c in range(NKT):
                    nc.tensor.matmul(
                        pv[:],
                        xt_t[:, c, m * 128 : (m + 1) * 128],
                        wv_sb[:, c, :],
                        start=(c == 0),
                        stop=(c == NKT - 1),
                    )
                g = t * (TT // 128) + m
                nc.scalar.copy(v_sb[:, g, :], pv[:])

        # ---- phase 2+3 interleaved: attention, then the output-projection
        # slice that just became ready, so out-proj matmuls fill the
        # ACT-bound bubbles of the attention chain ----
        PPT = QT // KC // 2  # score-pairs per q-tile step

        def attn(b, h, qt):
            q0 = b * S + qt * QT
            npair = (qt + 1) * PPT
            ov = psO.tile([128, QT], mybir.dt.float32, tag="ov")
            sm = psN.tile([128, QT], mybir.dt.float32, tag="sm")
            for p in range(npair):
                sc2 = psS.tile([128, 2 * QT], mybir.dt.float32, tag="sc")
                for cc in range(2):
                    k0 = b * S + (2 * p + cc) * KC
                    nc.tensor.matmul(
                        sc2[:, cc * QT : cc * QT + QT],
                        kr[:, h, k0 : k0 + KC],
                        qr[:, h, q0 : q0 + QT],
                        start=True,
                        stop=True,
                    )
                e2 = expp.tile([128, 2 * QT], bf16, tag="e")
                nc.scalar.activation(e2[:], sc2[:], Exp, scale=ISQRT)
                dd = 2 * p - qt * (QT // KC)
                if dd >= 0:
                    nc.vector.tensor_tensor(
                        e2[:], e2[:], m4_sb[:, dd * QT : dd * QT + 2 * QT], mult
                    )
                first = p == 0
                last = p == npair - 1
                for cc in range(2):
                    gk = (b * S + (2 * p + cc) * KC) // 128
                    nc.tensor.matmul(
                        ov[:],
                        v_sb[:, gk, h * HD : (h + 1) * HD],
                        e2[:, cc * QT : cc * QT + QT],
                        start=(first and cc == 0),
                        stop=(last and cc == 1),
                    )
                es = esp.tile([128, QT], bf16, tag="es")
                nc.vector.tensor_tensor(
                    es[:], e2[:, 0:QT], e2[:, QT : 2 * QT], add
                )
                nc.tensor.matmul(sm[:], ones_sb[:], es[:], start=first, stop=last)
            lnt = fpool.tile([128, QT], mybir.dt.float32, tag="f")
            nc.scalar.activation(lnt[:], sm[:], Ln)
            rr = fpool.tile([128, QT], mybir.dt.float32, tag="f")
            nc.scalar.activation(rr[:], lnt[:], Exp, scale=-1.0)
            # pre-touch rr on DVE so the norm TT only waits on PE
            nc.vector.tensor_copy(scratch[0:1, 4:6], rr[0:1, 0:2])
            nc.vector.tensor_tensor(
                on_sb[:, h, b, qt * QT : qt * QT + QT], ov[:], rr[:], mult
            )

        ecount = 0

        def outproj(b, qt):
            nonlocal ecount
            for tcn in range(4 * qt, 4 * qt + 4):
                for et in range(D // 512):
                    po = psA.tile([128, 512], mybir.dt.float32, tag="proj")
                    for j in range(HPC):
                        nc.tensor.matmul(
                            po[:],
                            on_sb[:, j, b, tcn * 128 : tcn * 128 + 128],
                            wo_sb[:, j, et * 512 : et * 512 + 512],
                            start=(j == 0),
                            stop=(j == HPC - 1),
                        )
                    ob = outp.tile([128, 512], mybir.dt.float32, tag="ob")
                    if ecount % 2 == 0:
                        nc.scalar.copy(ob[:], po[:])
                    else:
                        nc.vector.tensor_copy(ob[:], po[:])
                    ecount += 1
                    nc.sync.dma_start(
                        out_h[b * S + tcn * 128 : b * S + tcn * 128 + 128,
                              et * 512 : et * 512 + 512],
                        ob[:],
                    )

        for b in range(B):
            for qt in range(S // QT):
                for h in range(HPC):
                    attn(b, h, qt)
                outproj(b, qt)
    return _patch_serialization(nc)


def _prep_inputs(x, wq, wk, wv, wo, freqs_cos, freqs_sin):
    import ml_dtypes

    bf16 = ml_dtypes.bfloat16
    perm = np.concatenate([np.arange(0, HD, 2), np.arange(1, HD, 2)])

    xt = np.ascontiguousarray(x.reshape(BS, D).T)          # [D, BS]
    xt_r = np.ascontiguousarray(
        xt.reshape(NKT, 128, BS).transpose(1, 0, 2)
    ).astype(bf16)                                         # [128, NKT, BS]

    cosT = freqs_cos.T.astype(np.float32)                  # [64, S]
    sinT = freqs_sin.T.astype(np.float32)
    cs = np.concatenate(
        [np.tile(cosT, (1, 2 * B)), np.tile(sinT, (1, 2 * B))], axis=0
    ).astype(bf16)                                         # [128, 2*BS]

    i = np.arange(KC)[:, None]
    j = np.arange(QT)[None, :]
    m4 = np.concatenate(
        [(i + d <= j).astype(np.float32) for d in (0, 128, 256, 384)], axis=1
    ).astype(bf16)                                         # [128, 4*QT]

    def pack_w(wmat_cols):
        # wmat_cols: [D, 2*HD] -> [128, NKT, 2*HD]
        return np.ascontiguousarray(
            wmat_cols.reshape(NKT, 128, 2 * HD).transpose(1, 0, 2)
        ).astype(bf16)

    in_maps = []
    for c in range(NCORES):
        heads = [HPC * c + hh for hh in range(HPC)]
        wq_c = np.concatenate(
            [wq[h * HD : (h + 1) * HD][perm].T for h in heads], axis=1
        )                                                  # [D, 2*HD]
        wk_c = np.concatenate(
            [wk[h * HD : (h + 1) * HD][perm].T for h in heads], axis=1
        )
        wv_c = np.concatenate(
            [wv[h * HD : (h + 1) * HD].T for h in heads], axis=1
        )
        wo_c = np.stack(
            [wo[:, h * HD : (h + 1) * HD].T for h in heads], axis=0
        )                                                  # [2, HD, D]
        wo_r = np.ascontiguousarray(wo_c.transpose(1, 0, 2)).astype(bf16)  # [128,2,D]
        in_maps.append(
            dict(
                xt=xt_r,
                wq=pack_w(wq_c),
                wk=pack_w(wk_c),
                wv=pack_w(wv_c),
                wo=wo_r,
                cs=cs,
                m4=m4,
            )
        )
    return in_maps


_NC_CACHE = {}


def kernel(x, wq, wk, wv, wo, freqs_cos, freqs_sin, mask):
    from concourse.bass_utils import run_bass_kernel_spmd

    in_maps = _prep_inputs(x, wq, wk, wv, wo, freqs_cos, freqs_sin)
    if "nc" not in _NC_CACHE:
        _NC_CACHE["nc"] = _build_nc()
    nc = _NC_CACHE["nc"]
    res = run_bass_kernel_spmd(nc, in_maps, core_ids=list(range(NCORES)))
    parts = [r["out"].astype(np.float32) for r in res.results]
    out = np.sum(np.stack(parts, 0), axis=0, dtype=np.float32)
    return out.reshape(B, S, D)


# revision 14
# speedup vs baseline: 1.0168x; 1.0168x over previous
"""Distributed Trainium2 kernel for 16-head causal attention with RoPE.

B=2, S=2048, D=2048, H=16, HD=128. Tensor-parallel over heads: core c owns
heads {2c, 2c+1}. Each core computes q/k/v projections for its heads,
RoPE, causal attention, and a partial output projection (wo row-shard);
the host sums the 8 partials (the unshard step for a row-sharded wo).

Device-side layout choices (all transposes are done on the host):
  - x is fed pre-transposed as xt[d, tok] so every matmul contracts over
    the partition axis with no on-device transposes.
  - q/k are produced head-dim-major (qT[hd, tok]); the RoPE even/odd pair
    permutation is folded into the wq/wk columns on the host, so RoPE is
    six plain elementwise ops on [64, tok] slices.
  - scores are computed transposed (scoresT[k, q]); softmax sums over k
    (the partition axis) come from an all-ones [128,128] matmul that
    also broadcasts the sum to all partitions; 1/sum = exp(-ln(sum)).
  - attention output oT[hd, q] is exactly the lhsT the output projection
    needs, so the whole pipeline has zero on-device transposes.
"""

import numpy as np
from contextlib import ExitStack

B, S, D = 2, 2048, 2048
H, HD, HALF = 16, 128, 64
BS = B * S
NCORES = 8
HPC = H // NCORES          # heads per core
TT = 512                   # token tile for projections
QT = 512                   # q tile in attention
KC = 128                   # k chunk in attention
NKT = D // 128             # 16 contraction chunks of the model dim
NTT = BS // TT             # 8 token tiles
ISQRT = 1.0 / float(np.sqrt(HD))


def _legalize_waits(bir: bytes) -> bytes:
    """Split multi-wait sync_info into standalone EventSemaphore instructions.

    The neuronxcc walrus codegen only encodes ONE sync wait slot on compute
    instructions (Matmult/TensorTensor/...); Tile's sem-assignment freely
    emits several. Hoisting the extras into same-engine EventSemaphore
    instructions placed immediately before the consumer is semantically
    identical (the sequencer blocks on them in program order).
    """
    import json

    d = json.loads(bir)
    wid = 0
    for fn in d["functions"]:
        for blk in fn["blocks"]:
            out = []
            for inst in blk["instructions"]:
                si = inst.get("sync_info")
                if si:
                    waits = si.get("on_wait") or []
                    if len(waits) > 1 and inst.get("engine") not in (None, "Unassigned"):
                        for w in waits[:-1]:
                            wid += 1
                            out.append(
                                {
                                    "debug": inst.get("debug", 0),
                                    "engine": inst["engine"],
                                    "ins": [],
                                    "name": f"hoisted-wait-{wid}",
                                    "opcode": "EventSemaphore",
                                    "outs": [],
                                    "sync_info": {"on_update": [], "on_wait": [w]},
                                }
                            )
                        si["on_wait"] = [waits[-1]]
                out.append(inst)
            blk["instructions"] = out
    return json.dumps(d).encode()


def _patch_serialization(nc):
    import types

    orig = nc.to_json_bytes

    def patched(self):
        return _legalize_waits(orig())

    nc.to_json_bytes = types.MethodType(patched, nc)
    return nc


def _build_nc():
    import concourse.bass as bass
    import concourse.tile as tile
    from concourse import mybir

    f32 = mybir.dt.float32
    bf16 = mybir.dt.bfloat16
    Exp = mybir.ActivationFunctionType.Exp
    Ln = mybir.ActivationFunctionType.Ln
    mult = mybir.AluOpType.mult
    sub = mybir.AluOpType.subtract
    add = mybir.AluOpType.add

    nc = bass.Bass()

    xt_h = nc.declare_dram_parameter("xt", [128, NKT, BS], bf16, isOutput=False)
    wq_h = nc.declare_dram_parameter("wq", [128, NKT, 2 * HD], bf16, isOutput=False)
    wk_h = nc.declare_dram_parameter("wk", [128, NKT, 2 * HD], bf16, isOutput=False)
    wv_h = nc.declare_dram_parameter("wv", [128, NKT, 2 * HD], bf16, isOutput=False)
    wo_h = nc.declare_dram_parameter("wo", [128, 2, D], bf16, isOutput=False)
    cs_h = nc.declare_dram_parameter("cs", [128, 2 * BS], bf16, isOutput=False)
    m4_h = nc.declare_dram_parameter("m4", [128, 4 * QT], bf16, isOutput=False)
    out_h = nc.declare_dram_parameter("out", [BS, D], f32, isOutput=True)

    with ExitStack() as ctx:
        tc = ctx.enter_context(tile.TileContext(nc))
        const = ctx.enter_context(tc.tile_pool(name="const", bufs=1))
        persist = ctx.enter_context(tc.tile_pool(name="persist", bufs=1))
        xtp = ctx.enter_context(tc.tile_pool(name="xtp", bufs=2))
        expp = ctx.enter_context(tc.tile_pool(name="expp", bufs=4))
        esp = ctx.enter_context(tc.tile_pool(name="esp", bufs=4))
        ropet = ctx.enter_context(tc.tile_pool(name="ropet", bufs=8))
        fpool = ctx.enter_context(tc.tile_pool(name="fpool", bufs=3))
        outp = ctx.enter_context(tc.tile_pool(name="outp", bufs=4))
        psA = ctx.enter_context(tc.tile_pool(name="psA", bufs=2, space="PSUM"))
        psS = ctx.enter_context(tc.tile_pool(name="psS", bufs=2, space="PSUM"))
        psO = ctx.enter_context(tc.tile_pool(name="psO", bufs=1, space="PSUM"))
        psN = ctx.enter_context(tc.tile_pool(name="psN", bufs=1, space="PSUM"))

        # ---- constants into SBUF (fine-grained DMAs so the first
        # projection matmuls start as soon as their slices land) ----
        wq_sb = const.tile([128, NKT, 2 * HD], bf16, tag="wq")
        wk_sb = const.tile([128, NKT, 2 * HD], bf16, tag="wk")
        wv_sb = const.tile([128, NKT, 2 * HD], bf16, tag="wv")
        wo_sb = const.tile([128, 2, D], bf16, tag="wo")
        cs_sb = const.tile([128, 2 * BS], bf16, tag="cs")
        m4_sb = const.tile([128, 4 * QT], bf16, tag="m4")
        ones_sb = const.tile([128, 128], bf16, tag="ones")
        for c in range(NKT):
            nc.sync.dma_start(wq_sb[:, c, :], wq_h[:, c, :])
        CSC = 8  # cs load split so RoPE isn't gated on one 2 MB transfer
        for c in range(CSC):
            w = 2 * BS // CSC
            nc.sync.dma_start(cs_sb[:, c * w : (c + 1) * w], cs_h[:, c * w : (c + 1) * w])
        # first token tile ASAP — it gates the very first matmul
        xt_t0 = xtp.tile([128, NKT, TT], bf16, tag="xt")
        for c in range(NKT):
            nc.sync.dma_start(xt_t0[:, c, :], xt_h[:, c, 0:TT])
        for c in range(NKT):
            nc.sync.dma_start(wk_sb[:, c, :], wk_h[:, c, :])
        for c in range(NKT):
            nc.sync.dma_start(wv_sb[:, c, :], wv_h[:, c, :])
        nc.sync.dma_start(m4_sb[:], m4_h[:])
        nc.sync.dma_start(wo_sb[:], wo_h[:])
        nc.vector.memset(ones_sb[:], 1.0)

        # DVE pre-touch of DMA-written constants: TensorTensor instructions
        # encode only one sync-wait slot, so the DVE vector clock must have
        # observed these DMAs before any TT reads them (else walrus dies with
        # "Too many sync wait commands").
        scratch = const.tile([1, 8], bf16, tag="scratch")
        nc.vector.tensor_copy(scratch[0:1, 0:2], cs_sb[0:1, 0:2])
        nc.vector.tensor_copy(scratch[0:1, 2:4], m4_sb[0:1, 0:2])

        # persistent activations
        qr = persist.tile([128, HPC, BS], bf16, tag="qr")   # rotated qT per head
        kr = persist.tile([128, HPC, BS], bf16, tag="kr")   # rotated kT per head
        v_sb = persist.tile([128, BS // 128, 2 * HD], bf16, tag="v")  # tok-major v
        on_sb = persist.tile([128, HPC, B, S], bf16, tag="on")  # normalized oT

        # ---- phase 1: projections + RoPE ----
        for t in range(NTT):
            t0 = t * TT
            if t == 0:
                xt_t = xt_t0
            else:
                xt_t = xtp.tile([128, NKT, TT], bf16, tag="xt")
                for c in range(NKT):
                    nc.sync.dma_start(xt_t[:, c, :], xt_h[:, c, t0 : t0 + TT])

            for h in range(HPC):
                for w_sb, dstT in ((wq_sb, qr), (wk_sb, kr)):
                    pq = psA.tile([128, TT], mybir.dt.float32, tag="proj")
                    for c in range(NKT):
                        nc.tensor.matmul(
                            pq[:],
                            w_sb[:, c, h * HD : (h + 1) * HD],
                            xt_t[:, c, :],
                            start=(c == 0),
                            stop=(c == NKT - 1),
                        )
                    co = cs_sb[0:HALF, h * BS + t0 : h * BS + t0 + TT]
                    si = cs_sb[HALF:128, h * BS + t0 : h * BS + t0 + TT]
                    t1 = ropet.tile([HALF, TT], bf16, tag="rt")
                    t2 = ropet.tile([HALF, TT], bf16, tag="rt")
                    t3 = ropet.tile([HALF, TT], bf16, tag="rt")
                    t4 = ropet.tile([HALF, TT], bf16, tag="rt")
                    nc.vector.tensor_tensor(t1[:], pq[0:HALF, :], co, mult)
                    nc.vector.tensor_tensor(t2[:], pq[HALF:128, :], si, mult)
                    nc.vector.tensor_tensor(
                        dstT[0:HALF, h, t0 : t0 + TT], t1[:], t2[:], sub
                    )
                    nc.vector.tensor_tensor(t3[:], pq[0:HALF, :], si, mult)
                    nc.vector.tensor_tensor(t4[:], pq[HALF:128, :], co, mult)
                    nc.vector.tensor_tensor(
                        dstT[HALF:128, h, t0 : t0 + TT], t3[:], t4[:], add
                    )

            # v projection, token-major [tok, 2*HD]
            for m in range(TT // 128):
                pv = psA.tile([128, 2 * HD], mybir.dt.float32, tag="proj")
                for c in range(NKT):
                    nc.tensor.matmul(
                        pv[:],
                        xt_t[:, c, m * 128 : (m + 1) * 128],
                        wv_sb[:, c, :],
                        start=(c == 0),
                        stop=(c == NKT - 1),
                    )
                g = t * (TT // 128) + m
                nc.scalar.copy(v_sb[:, g, :], pv[:])

        # ---- phase 2+3 interleaved: attention, then the output-projection
        # slice that just became ready, so out-proj matmuls fill the
        # ACT-bound bubbles of the attention chain ----
        PPT = QT // KC // 2  # score-pairs per q-tile step

        def attn(b, h, qt):
            q0 = b * S + qt * QT
            npair = (qt + 1) * PPT
            ov = psO.tile([128, QT], mybir.dt.float32, tag="ov")
            sm = psN.tile([128, QT], mybir.dt.float32, tag="sm")
            for p in range(npair):
                sc2 = psS.tile([128, 2 * QT], mybir.dt.float32, tag="sc")
                for cc in range(2):
                    k0 = b * S + (2 * p + cc) * KC
                    nc.tensor.matmul(
                        sc2[:, cc * QT : cc * QT + QT],
                        kr[:, h, k0 : k0 + KC],
                        qr[:, h, q0 : q0 + QT],
                        start=True,
                        stop=True,
                    )
                e2 = expp.tile([128, 2 * QT], bf16, tag="e")
                nc.scalar.activation(e2[:], sc2[:], Exp, scale=ISQRT)
                dd = 2 * p - qt * (QT // KC)
                if dd >= 0:
                    nc.vector.tensor_tensor(
                        e2[:], e2[:], m4_sb[:, dd * QT : dd * QT + 2 * QT], mult
                    )
                first = p == 0
                last = p == npair - 1
                for cc in range(2):
                    gk = (b * S + (2 * p + cc) * KC) // 128
                    nc.tensor.matmul(
                        ov[:],
                        v_sb[:, gk, h * HD : (h + 1) * HD],
                        e2[:, cc * QT : cc * QT + QT],
                        start=(first and cc == 0),
                        stop=(last and cc == 1),
                    )
                es = esp.tile([128, QT], bf16, tag="es")
                nc.vector.tensor_tensor(
                    es[:], e2[:, 0:QT], e2[:, QT : 2 * QT], add
                )
                nc.tensor.matmul(sm[:], ones_sb[:], es[:], start=first, stop=last)
            lnt = fpool.tile([128, QT], mybir.dt.float32, tag="f")
            nc.scalar.activation(lnt[:], sm[:], Ln)
            rr = fpool.tile([128, QT], mybir.dt.float32, tag="f")
            nc.scalar.activation(rr[:], lnt[:], Exp, scale=-1.0)
            # pre-touch rr on DVE so the norm TT only waits on PE
            nc.vector.tensor_copy(scratch[0:1, 4:6], rr[0:1, 0:2])
            nc.vector.tensor_tensor(
                on_sb[:, h, b, qt * QT : qt * QT + QT], ov[:], rr[:], mult
            )

        ecount = 0

        def outproj(b, qt):
            nonlocal ecount
            for tcn in range(4 * qt, 4 * qt + 4):
                for et in range(D // 512):
                    po = psA.tile([128, 512], mybir.dt.float32, tag="proj")
                    for j in range(HPC):
                        nc.tensor.matmul(
                            po[:],
                            on_sb[:, j, b, tcn * 128 : tcn * 128 + 128],
                            wo_sb[:, j, et * 512 : et * 512 + 512],
                            start=(j == 0),
                            stop=(j == HPC - 1),
                        )
                    ob = outp.tile([128, 512], mybir.dt.float32, tag="ob")
                    if ecount % 2 == 0:
                        nc.scalar.copy(ob[:], po[:])
                    else:
                        nc.vector.tensor_copy(ob[:], po[:])
                    ecount += 1
                    # stores go out via SWDGE (gpsimd) to keep SP free
                    # for the latency-critical load path
                    nc.gpsimd.dma_start(
                        out_h[b * S + tcn * 128 : b * S + tcn * 128 + 128,
                              et * 512 : et * 512 + 512],
                        ob[:],
                    )

        for b in range(B):
            for qt in range(S // QT):
                for h in range(HPC):
                    attn(b, h, qt)
                outproj(b, qt)
    return _patch_serialization(nc)


def _prep_inputs(x, wq, wk, wv, wo, freqs_cos, freqs_sin):
    import ml_dtypes

    bf16 = ml_dtypes.bfloat16
    perm = np.concatenate([np.arange(0, HD, 2), np.arange(1, HD, 2)])

    xt = np.ascontiguousarray(x.reshape(BS, D).T)          # [D, BS]
    xt_r = np.ascontiguousarray(
        xt.reshape(NKT, 128, BS).transpose(1, 0, 2)
    ).astype(bf16)                                         # [128, NKT, BS]

    cosT = freqs_cos.T.astype(np.float32)                  # [64, S]
    sinT = freqs_sin.T.astype(np.float32)
    cs = np.concatenate(
        [np.tile(cosT, (1, 2 * B)), np.tile(sinT, (1, 2 * B))], axis=0
    ).astype(bf16)                                         # [128, 2*BS]

    i = np.arange(KC)[:, None]
    j = np.arange(QT)[None, :]
    m4 = np.concatenate(
        [(i + d <= j).astype(np.float32) for d in (0, 128, 256, 384)], axis=1
    ).astype(bf16)                                         # [128, 4*QT]

    def pack_w(wmat_cols):
        # wmat_cols: [D, 2*HD] -> [128, NKT, 2*HD]
        return np.ascontiguousarray(
            wmat_cols.reshape(NKT, 128, 2 * HD).transpose(1, 0, 2)
        ).astype(bf16)

    in_maps = []
    for c in range(NCORES):
        heads = [HPC * c + hh for hh in range(HPC)]
        wq_c = np.concatenate(
            [wq[h * HD : (h + 1) * HD][perm].T for h in heads], axis=1
        )                                                  # [D, 2*HD]
        wk_c = np.concatenate(
            [wk[h * HD : (h + 1) * HD][perm].T for h in heads], axis=1
        )
        wv_c = np.concatenate(
            [wv[h * HD : (h + 1) * HD].T for h in heads], axis=1
        )
        wo_c = np.stack(
            [wo[:, h * HD : (h + 1) * HD].T for h in heads], axis=0
        )                                                  # [2, HD, D]
        wo_r = np.ascontiguousarray(wo_c.transpose(1, 0, 2)).astype(bf16)  # [128,2,D]
        in_maps.append(
            dict(
                xt=xt_r,
                wq=pack_w(wq_c),
                wk=pack_w(wk_c),
                wv=pack_w(wv_c),
                wo=wo_r,
                cs=cs,
                m4=m4,
            )
        )
    return in_maps


_NC_CACHE = {}


def kernel(x, wq, wk, wv, wo, freqs_cos, freqs_sin, mask):
    from concourse.bass_utils import run_bass_kernel_spmd

    in_maps = _prep_inputs(x, wq, wk, wv, wo, freqs_cos, freqs_sin)
    if "nc" not in _NC_CACHE:
        _NC_CACHE["nc"] = _build_nc()
    nc = _NC_CACHE["nc"]
    res = run_bass_kernel_spmd(nc, in_maps, core_ids=list(range(NCORES)))
    parts = [r["out"].astype(np.float32) for r in res.results]
    out = np.sum(np.stack(parts, 0), axis=0, dtype=np.float32)
    return out.reshape(B, S, D)


# revision 17
# speedup vs baseline: 1.0678x; 1.0501x over previous
"""Distributed Trainium2 kernel for 16-head causal attention with RoPE.

B=2, S=2048, D=2048, H=16, HD=128. Tensor-parallel over heads: core c owns
heads {2c, 2c+1}. Each core computes q/k/v projections for its heads,
RoPE, causal attention, and a partial output projection (wo row-shard);
the host sums the 8 partials (the unshard step for a row-sharded wo).

Device-side layout choices (all transposes are done on the host):
  - x is fed pre-transposed as xt[d, tok] so every matmul contracts over
    the partition axis with no on-device transposes.
  - q/k are produced head-dim-major (qT[hd, tok]); the RoPE even/odd pair
    permutation is folded into the wq/wk columns on the host, so RoPE is
    six plain elementwise ops on [64, tok] slices.
  - scores are computed transposed (scoresT[k, q]); softmax sums over k
    (the partition axis) come from an all-ones [128,128] matmul that
    also broadcasts the sum to all partitions; 1/sum = exp(-ln(sum)).
  - attention output oT[hd, q] is exactly the lhsT the output projection
    needs, so the whole pipeline has zero on-device transposes.
"""

import numpy as np
from contextlib import ExitStack

B, S, D = 2, 2048, 2048
H, HD, HALF = 16, 128, 64
BS = B * S
NCORES = 8
HPC = H // NCORES          # heads per core
TT = 512                   # token tile for projections
QT = 512                   # q tile in attention
KC = 128                   # k chunk in attention
NKT = D // 128             # 16 contraction chunks of the model dim
NTT = BS // TT             # 8 token tiles
ISQRT = 1.0 / float(np.sqrt(HD))


def _legalize_waits(bir: bytes) -> bytes:
    """Split multi-wait sync_info into standalone EventSemaphore instructions.

    The neuronxcc walrus codegen only encodes ONE sync wait slot on compute
    instructions (Matmult/TensorTensor/...); Tile's sem-assignment freely
    emits several. Hoisting the extras into same-engine EventSemaphore
    instructions placed immediately before the consumer is semantically
    identical (the sequencer blocks on them in program order).
    """
    import json

    d = json.loads(bir)
    wid = 0
    for fn in d["functions"]:
        for blk in fn["blocks"]:
            out = []
            for inst in blk["instructions"]:
                si = inst.get("sync_info")
                if si:
                    waits = si.get("on_wait") or []
                    if len(waits) > 1 and inst.get("engine") not in (None, "Unassigned"):
                        for w in waits[:-1]:
                            wid += 1
                            out.append(
                                {
                                    "debug": inst.get("debug", 0),
                                    "engine": inst["engine"],
                                    "ins": [],
                                    "name": f"hoisted-wait-{wid}",
                                    "opcode": "EventSemaphore",
                                    "outs": [],
                                    "sync_info": {"on_update": [], "on_wait": [w]},
                                }
                            )
                        si["on_wait"] = [waits[-1]]
                out.append(inst)
            blk["instructions"] = out
    return json.dumps(d).encode()


def _patch_serialization(nc):
    import types

    orig = nc.to_json_bytes

    def patched(self):
        return _legalize_waits(orig())

    nc.to_json_bytes = types.MethodType(patched, nc)
    return nc


def _build_nc():
    import concourse.bass as bass
    import concourse.tile as tile
    from concourse import mybir

    f32 = mybir.dt.float32
    bf16 = mybir.dt.bfloat16
    Exp = mybir.ActivationFunctionType.Exp
    Ln = mybir.ActivationFunctionType.Ln
    mult = mybir.AluOpType.mult
    sub = mybir.AluOpType.subtract
    add = mybir.AluOpType.add

    nc = bass.Bass()

    xt_h = nc.declare_dram_parameter("xt", [128, NKT, BS], bf16, isOutput=False)
    wq_h = nc.declare_dram_parameter("wq", [128, NKT, 2 * HD], bf16, isOutput=False)
    wk_h = nc.declare_dram_parameter("wk", [128, NKT, 2 * HD], bf16, isOutput=False)
    wv_h = nc.declare_dram_parameter("wv", [128, NKT, 2 * HD], bf16, isOutput=False)
    wo_h = nc.declare_dram_parameter("wo", [128, 2, D], bf16, isOutput=False)
    cs_h = nc.declare_dram_parameter("cs", [128, 2 * BS], bf16, isOutput=False)
    m4_h = nc.declare_dram_parameter("m4", [128, 4 * QT], bf16, isOutput=False)
    out_h = nc.declare_dram_parameter("out", [BS, D], f32, isOutput=True)

    with ExitStack() as ctx:
        tc = ctx.enter_context(tile.TileContext(nc))
        const = ctx.enter_context(tc.tile_pool(name="const", bufs=1))
        persist = ctx.enter_context(tc.tile_pool(name="persist", bufs=1))
        xtp = ctx.enter_context(tc.tile_pool(name="xtp", bufs=2))
        expp = ctx.enter_context(tc.tile_pool(name="expp", bufs=4))
        esp = ctx.enter_context(tc.tile_pool(name="esp", bufs=4))
        ropet = ctx.enter_context(tc.tile_pool(name="ropet", bufs=8))
        fpool = ctx.enter_context(tc.tile_pool(name="fpool", bufs=3))
        outp = ctx.enter_context(tc.tile_pool(name="outp", bufs=4))
        psA = ctx.enter_context(tc.tile_pool(name="psA", bufs=2, space="PSUM"))
        psS = ctx.enter_context(tc.tile_pool(name="psS", bufs=2, space="PSUM"))
        psO = ctx.enter_context(tc.tile_pool(name="psO", bufs=1, space="PSUM"))
        psN = ctx.enter_context(tc.tile_pool(name="psN", bufs=1, space="PSUM"))

        # ---- constants into SBUF (fine-grained DMAs so the first
        # projection matmuls start as soon as their slices land) ----
        wq_sb = const.tile([128, NKT, 2 * HD], bf16, tag="wq")
        wk_sb = const.tile([128, NKT, 2 * HD], bf16, tag="wk")
        wv_sb = const.tile([128, NKT, 2 * HD], bf16, tag="wv")
        wo_sb = const.tile([128, 2, D], bf16, tag="wo")
        cs_sb = const.tile([128, 2 * BS], bf16, tag="cs")
        m4_sb = const.tile([128, 4 * QT], bf16, tag="m4")
        ones_sb = const.tile([128, 128], bf16, tag="ones")
        # DMA issue costs ~0.6us per dma_start on a sequencer, so use
        # medium-grained transfers, split across BOTH hwdge engines
        # (sync + scalar issue in parallel), most-urgent first.
        for c in range(4):  # wq in 4 × 256KB on sync
            nc.sync.dma_start(wq_sb[:, 4 * c : 4 * c + 4, :], wq_h[:, 4 * c : 4 * c + 4, :])
        # first token tile gates the very first matmul: issue on scalar
        xt_t0 = xtp.tile([128, NKT, TT], bf16, tag="xt")
        for c in range(4):
            nc.scalar.dma_start(xt_t0[:, 4 * c : 4 * c + 4, :], xt_h[:, 4 * c : 4 * c + 4, 0:TT])
        for c in range(4):  # cs (RoPE needs it right after first projection)
            w = 2 * BS // 4
            nc.sync.dma_start(cs_sb[:, c * w : (c + 1) * w], cs_h[:, c * w : (c + 1) * w])
        for c in range(4):
            nc.scalar.dma_start(wk_sb[:, 4 * c : 4 * c + 4, :], wk_h[:, 4 * c : 4 * c + 4, :])
        for c in range(2):
            nc.sync.dma_start(wv_sb[:, 8 * c : 8 * c + 8, :], wv_h[:, 8 * c : 8 * c + 8, :])
        nc.sync.dma_start(m4_sb[:], m4_h[:])
        nc.sync.dma_start(wo_sb[:], wo_h[:])
        nc.vector.memset(ones_sb[:], 1.0)

        # DVE pre-touch of DMA-written constants: TensorTensor instructions
        # encode only one sync-wait slot, so the DVE vector clock must have
        # observed these DMAs before any TT reads them (else walrus dies with
        # "Too many sync wait commands").
        scratch = const.tile([1, 8], bf16, tag="scratch")
        nc.vector.tensor_copy(scratch[0:1, 0:2], cs_sb[0:1, 0:2])
        nc.vector.tensor_copy(scratch[0:1, 2:4], m4_sb[0:1, 0:2])

        # persistent activations
        qr = persist.tile([128, HPC, BS], bf16, tag="qr")   # rotated qT per head
        kr = persist.tile([128, HPC, BS], bf16, tag="kr")   # rotated kT per head
        v_sb = persist.tile([128, BS // 128, 2 * HD], bf16, tag="v")  # tok-major v
        on_sb = persist.tile([128, HPC, B, S], bf16, tag="on")  # normalized oT

        # ---- phase 1: projections + RoPE ----
        for t in range(NTT):
            t0 = t * TT
            if t == 0:
                xt_t = xt_t0
            else:
                xt_t = xtp.tile([128, NKT, TT], bf16, tag="xt")
                dma_eng = nc.sync if t % 2 else nc.scalar
                for c in range(4):
                    dma_eng.dma_start(
                        xt_t[:, 4 * c : 4 * c + 4, :],
                        xt_h[:, 4 * c : 4 * c + 4, t0 : t0 + TT],
                    )

            for h in range(HPC):
                for w_sb, dstT in ((wq_sb, qr), (wk_sb, kr)):
                    pq = psA.tile([128, TT], mybir.dt.float32, tag="proj")
                    for c in range(NKT):
                        nc.tensor.matmul(
                            pq[:],
                            w_sb[:, c, h * HD : (h + 1) * HD],
                            xt_t[:, c, :],
                            start=(c == 0),
                            stop=(c == NKT - 1),
                        )
                    co = cs_sb[0:HALF, h * BS + t0 : h * BS + t0 + TT]
                    si = cs_sb[HALF:128, h * BS + t0 : h * BS + t0 + TT]
                    t1 = ropet.tile([HALF, TT], bf16, tag="rt")
                    t2 = ropet.tile([HALF, TT], bf16, tag="rt")
                    t3 = ropet.tile([HALF, TT], bf16, tag="rt")
                    t4 = ropet.tile([HALF, TT], bf16, tag="rt")
                    nc.vector.tensor_tensor(t1[:], pq[0:HALF, :], co, mult)
                    nc.vector.tensor_tensor(t2[:], pq[HALF:128, :], si, mult)
                    nc.vector.tensor_tensor(
                        dstT[0:HALF, h, t0 : t0 + TT], t1[:], t2[:], sub
                    )
                    nc.vector.tensor_tensor(t3[:], pq[0:HALF, :], si, mult)
                    nc.vector.tensor_tensor(t4[:], pq[HALF:128, :], co, mult)
                    nc.vector.tensor_tensor(
                        dstT[HALF:128, h, t0 : t0 + TT], t3[:], t4[:], add
                    )

            # v projection, token-major [tok, 2*HD]
            for m in range(TT // 128):
                pv = psA.tile([128, 2 * HD], mybir.dt.float32, tag="proj")
                for c in range(NKT):
                    nc.tensor.matmul(
                        pv[:],
                        xt_t[:, c, m * 128 : (m + 1) * 128],
                        wv_sb[:, c, :],
                        start=(c == 0),
                        stop=(c == NKT - 1),
                    )
                g = t * (TT // 128) + m
                nc.scalar.copy(v_sb[:, g, :], pv[:])

        # ---- phase 2+3 interleaved: attention, then the output-projection
        # slice that just became ready, so out-proj matmuls fill the
        # ACT-bound bubbles of the attention chain ----
        PPT = QT // KC // 2  # score-pairs per q-tile step

        def attn(b, h, qt):
            q0 = b * S + qt * QT
            npair = (qt + 1) * PPT
            ov = psO.tile([128, QT], mybir.dt.float32, tag="ov")
            sm = psN.tile([128, QT], mybir.dt.float32, tag="sm")
            for p in range(npair):
                sc2 = psS.tile([128, 2 * QT], mybir.dt.float32, tag="sc")
                for cc in range(2):
                    k0 = b * S + (2 * p + cc) * KC
                    nc.tensor.matmul(
                        sc2[:, cc * QT : cc * QT + QT],
                        kr[:, h, k0 : k0 + KC],
                        qr[:, h, q0 : q0 + QT],
                        start=True,
                        stop=True,
                    )
                e2 = expp.tile([128, 2 * QT], bf16, tag="e")
                nc.scalar.activation(e2[:], sc2[:], Exp, scale=ISQRT)
                dd = 2 * p - qt * (QT // KC)
                if dd >= 0:
                    nc.vector.tensor_tensor(
                        e2[:], e2[:], m4_sb[:, dd * QT : dd * QT + 2 * QT], mult
                    )
                first = p == 0
                last = p == npair - 1
                for cc in range(2):
                    gk = (b * S + (2 * p + cc) * KC) // 128
                    nc.tensor.matmul(
                        ov[:],
                        v_sb[:, gk, h * HD : (h + 1) * HD],
                        e2[:, cc * QT : cc * QT + QT],
                        start=(first and cc == 0),
                        stop=(last and cc == 1),
                    )
                es = esp.tile([128, QT], bf16, tag="es")
                nc.vector.tensor_tensor(
                    es[:], e2[:, 0:QT], e2[:, QT : 2 * QT], add
                )
                nc.tensor.matmul(sm[:], ones_sb[:], es[:], start=first, stop=last)
            lnt = fpool.tile([128, QT], mybir.dt.float32, tag="f")
            nc.scalar.activation(lnt[:], sm[:], Ln)
            rr = fpool.tile([128, QT], mybir.dt.float32, tag="f")
            nc.scalar.activation(rr[:], lnt[:], Exp, scale=-1.0)
            # pre-touch rr on DVE so the norm TT only waits on PE
            nc.vector.tensor_copy(scratch[0:1, 4:6], rr[0:1, 0:2])
            nc.vector.tensor_tensor(
                on_sb[:, h, b, qt * QT : qt * QT + QT], ov[:], rr[:], mult
            )

        ecount = 0

        def outproj(b, qt):
            nonlocal ecount
            for tcn in range(4 * qt, 4 * qt + 4):
                for et in range(D // 512):
                    po = psA.tile([128, 512], mybir.dt.float32, tag="proj")
                    for j in range(HPC):
                        nc.tensor.matmul(
                            po[:],
                            on_sb[:, j, b, tcn * 128 : tcn * 128 + 128],
                            wo_sb[:, j, et * 512 : et * 512 + 512],
                            start=(j == 0),
                            stop=(j == HPC - 1),
                        )
                    ob = outp.tile([128, 512], mybir.dt.float32, tag="ob")
                    if ecount % 2 == 0:
                        nc.scalar.copy(ob[:], po[:])
                    else:
                        nc.vector.tensor_copy(ob[:], po[:])
                    ecount += 1
                    # stores go out via SWDGE (gpsimd) to keep SP free
                    # for the latency-critical load path
                    nc.gpsimd.dma_start(
                        out_h[b * S + tcn * 128 : b * S + tcn * 128 + 128,
                              et * 512 : et * 512 + 512],
                        ob[:],
                    )

        # out-proj for q-tile qt-1 is emitted in the middle of q-tile qt's
        # attention: its inputs are then long-ready, so the PE never stalls
        # on the fresh sum->ln->exp->norm chain of the tile it just finished.
        pending = None
        for b in range(B):
            for qt in range(S // QT):
                attn(b, 0, qt)
                if pending is not None:
                    outproj(*pending)
                attn(b, 1, qt)
                pending = (b, qt)
        outproj(*pending)
    return _patch_serialization(nc)


def _prep_inputs(x, wq, wk, wv, wo, freqs_cos, freqs_sin):
    import ml_dtypes

    bf16 = ml_dtypes.bfloat16
    perm = np.concatenate([np.arange(0, HD, 2), np.arange(1, HD, 2)])

    xt = np.ascontiguousarray(x.reshape(BS, D).T)          # [D, BS]
    xt_r = np.ascontiguousarray(
        xt.reshape(NKT, 128, BS).transpose(1, 0, 2)
    ).astype(bf16)                                         # [128, NKT, BS]

    cosT = freqs_cos.T.astype(np.float32)                  # [64, S]
    sinT = freqs_sin.T.astype(np.float32)
    cs = np.concatenate(
        [np.tile(cosT, (1, 2 * B)), np.tile(sinT, (1, 2 * B))], axis=0
    ).astype(bf16)                                         # [128, 2*BS]

    i = np.arange(KC)[:, None]
    j = np.arange(QT)[None, :]
    m4 = np.concatenate(
        [(i + d <= j).astype(np.float32) for d in (0, 128, 256, 384)], axis=1
    ).astype(bf16)                                         # [128, 4*QT]

    def pack_w(wmat_cols):
        # wmat_cols: [D, 2*HD] -> [128, NKT, 2*HD]
        return np.ascontiguousarray(
            wmat_cols.reshape(NKT, 128, 2 * HD).transpose(1, 0, 2)
        ).astype(bf16)

    in_maps = []
    for c in range(NCORES):
        heads = [HPC * c + hh for hh in range(HPC)]
        wq_c = np.concatenate(
            [wq[h * HD : (h + 1) * HD][perm].T for h in heads], axis=1
        )                                                  # [D, 2*HD]
        wk_c = np.concatenate(
            [wk[h * HD : (h + 1) * HD][perm].T for h in heads], axis=1
        )
        wv_c = np.concatenate(
            [wv[h * HD : (h + 1) * HD].T for h in heads], axis=1
        )
        wo_c = np.stack(
            [wo[:, h * HD : (h + 1) * HD].T for h in heads], axis=0
        )                                                  # [2, HD, D]
        wo_r = np.ascontiguousarray(wo_c.transpose(1, 0, 2)).astype(bf16)  # [128,2,D]
        in_maps.append(
            dict(
                xt=xt_r,
                wq=pack_w(wq_c),
                wk=pack_w(wk_c),
                wv=pack_w(wv_c),
                wo=wo_r,
                cs=cs,
                m4=m4,
            )
        )
    return in_maps


_NC_CACHE = {}


def kernel(x, wq, wk, wv, wo, freqs_cos, freqs_sin, mask):
    from concourse.bass_utils import run_bass_kernel_spmd

    in_maps = _prep_inputs(x, wq, wk, wv, wo, freqs_cos, freqs_sin)
    if "nc" not in _NC_CACHE:
        _NC_CACHE["nc"] = _build_nc()
    nc = _NC_CACHE["nc"]
    res = run_bass_kernel_spmd(nc, in_maps, core_ids=list(range(NCORES)))
    parts = [r["out"].astype(np.float32) for r in res.results]
    out = np.sum(np.stack(parts, 0), axis=0, dtype=np.float32)
    return out.reshape(B, S, D)
